# revision 1
# baseline (speedup 1.0000x reference)
"""2-layer GraphSAGE (mean aggr + BN(eval) + ReLU) on Trainium2, 8-core SPMD.

Strategy (graph/data parallel, dst-node sharding):
  - Host: sort edges by dst, partition dst nodes into 8 ranges (49 chunks of
    128 nodes per core). Within each chunk, split edges into lo (src < 32768)
    and hi (src >= 32768) streams (dma_gather indices are signed int16), pad
    each stream to 128-edge tiles with per-chunk tile counts shared across
    cores (SPMD: one program, 8 cores). Gather indices are pre-wrapped into
    the dma_gather layout (idx i at [16g + i%16, i//16], 8 group replicas).
  - Device layer 1 (per core, per 128-node chunk):
      dma_gather x rows (lo stream from x_lo table, hi from x_hi table,
      batched GK tiles per call, batches span chunk boundaries)
      build selection S[e, j] = (dstloc[e]==j) * invdeg[dst[e]] on DVE
      PSUM aggT[ch, node] += G^T @ S  (PE)           == (D^-1 A x)^T chunk
      h^T = relu(W1l'^T @ aggT + W1r'^T @ xT + c1)   (BN folded into W/c)
      also emit hW2l = (h @ W2_l) row-major for layer-2 gathers
  - Host: allgather hW2l parts (concat + split at 32768), relaunch.
  - Device layer 2: same aggregation over hW2l rows (64ch), + W2_r self term
    + b2, transpose to row-major out.
"""

import numpy as np

import concourse.bacc as bacc
import concourse.bass as bass
import concourse.mybir as mybir
import concourse.tile as tile
from concourse.bass_utils import run_bass_kernel_spmd

F32 = mybir.dt.float32
I16 = mybir.dt.int16
AF = mybir.ActivationFunctionType
OP = mybir.AluOpType

N_CORES = 8
P = 128
SPLIT = 32768                           # int16 signed index limit


class Cfg:
    def __init__(self, n_nodes, c_in, c_hid, c_out, cpc):
        self.N = n_nodes
        self.C, self.H, self.O = c_in, c_hid, c_out
        self.CPC = cpc                  # 128-node chunks per core
        self.NPC = cpc * P              # nodes per core
        self.NP = self.NPC * N_CORES    # padded node count
        assert self.NP >= n_nodes
        self.NLO = min(self.NP, SPLIT)          # rows in lo table
        self.NHI = max(self.NP - SPLIT, 1)      # rows in hi table


CFG = Cfg(50000, 128, 128, 64, 49)      # NP = 50176
GK = 8                                  # edge tiles per dma_gather call
                                        # (>=2048 idxs per call crashes HW)


def _wrap_idx(a):
    """[NC, 128, NT] int32 -> dma_gather wrapped [NC, 128, NT*8] int16.
    Per tile t, edge e: wrapped[16g + e%16, t*8 + e//16] = a[e, t]."""
    ncr, _, nt = a.shape
    w16 = (a.reshape(ncr, 8, 16, nt).transpose(0, 2, 3, 1)
           .reshape(ncr, 16, nt * 8))
    return np.tile(w16, (1, 8, 1)).astype(np.uint16).view(np.int16)


def _preprocess(edge_index, cfg):
    """Sort/partition edges; split per chunk into lo/hi gather streams.
    Returns per-chunk tile counts (shared across cores), wrapped int16 index
    arrays, and [128, NT_lo+NT_hi] ids/ivd tile arrays per core."""
    src = np.asarray(edge_index[0]).astype(np.int64)
    dst = np.asarray(edge_index[1]).astype(np.int64)
    order = np.argsort(dst, kind="stable")
    s_src = src[order].astype(np.int32)
    s_dst = dst[order].astype(np.int32)
    deg = np.bincount(dst, minlength=cfg.NP).astype(np.float32)
    invdeg = (1.0 / np.maximum(deg, 1.0)).astype(np.float32)
    bounds = np.searchsorted(s_dst, np.arange(0, cfg.NP + 1, P)).astype(np.int64)
    islo = s_src < SPLIT

    nlo = np.zeros((N_CORES, cfg.CPC), np.int64)
    nhi = np.zeros((N_CORES, cfg.CPC), np.int64)
    for c in range(N_CORES):
        for ci in range(cfg.CPC):
            g = c * cfg.CPC + ci
            e0, e1 = int(bounds[g]), int(bounds[g + 1])
            nlo[c, ci] = int(islo[e0:e1].sum())
            nhi[c, ci] = (e1 - e0) - nlo[c, ci]
    T_lo = ((nlo.max(axis=0) + P - 1) // P).astype(np.int64)
    T_hi = ((nhi.max(axis=0) + P - 1) // P).astype(np.int64)
    T_lo[(T_lo == 0) & (T_hi == 0)] = 1     # every chunk needs >=1 matmul
    NT_lo, NT_hi = int(T_lo.sum()), int(T_hi.sum())
    lo_start = np.zeros(cfg.CPC, np.int64)
    lo_start[1:] = np.cumsum(T_lo)[:-1]
    hi_start = np.zeros(cfg.CPC, np.int64)
    hi_start[1:] = np.cumsum(T_hi)[:-1]
    NTT = NT_lo + NT_hi

    src_lo = np.zeros((N_CORES, P, max(NT_lo, 1)), np.int32)
    src_hi = np.zeros((N_CORES, P, max(NT_hi, 1)), np.int32)
    ids = np.full((N_CORES, P, NTT), 300.0, np.float32)
    ivd = np.zeros((N_CORES, P, NTT), np.float32)
    for c in range(N_CORES):
        for ci in range(cfg.CPC):
            g = c * cfg.CPC + ci
            e0, e1 = int(bounds[g]), int(bounds[g + 1])
            if e1 == e0:
                continue
            es = s_src[e0:e1]
            ed = s_dst[e0:e1]
            m = es < SPLIT
            for sel, st, s_off, arr, base in (
                (m, lo_start, 0, src_lo, 0),
                (~m, hi_start, NT_lo, src_hi, SPLIT),
            ):
                vs = es[sel] - base
                vd = ed[sel]
                n = len(vs)
                if n == 0:
                    continue
                j = np.arange(n)
                t = st[ci] + j // P
                pp = j % P
                arr[c, pp, t] = vs
                ids[c, pp, s_off + t] = (vd - (c * cfg.NPC + ci * P)).astype(np.float32)
                ivd[c, pp, s_off + t] = invdeg[vd]
    return T_lo, T_hi, _wrap_idx(src_lo), _wrap_idx(src_hi), ids, ivd


def _mk_nc():
    return bacc.Bacc(
        "TRN2",
        target_bir_lowering=False,
        debug=False,
        enable_asserts=False,
        num_devices=N_CORES,
    )


def _agg_chunks(nc, cfg, T_lo, T_hi, d_lotab, d_hitab, t_idxlo, t_idxhi,
                t_ids, t_ivd, t_iota, gplo, gphi, sp, pA, width, tail,
                close_group=True):
    """Shared aggregation loop: per chunk accumulate G^T @ S into PSUM over
    the lo then hi tile streams, then call tail(ci, ps_agg). With
    close_group=False the accumulation group stays open for tail to finish."""
    NT_lo = int(T_lo.sum())
    NT_hi = int(T_hi.sum())
    state = {"lo": [0, None, 0], "hi": [0, None, 0]}  # cursor, tile, base
    lo_pos = hi_pos = 0
    for ci in range(cfg.CPC):
        Tl, Th = int(T_lo[ci]), int(T_hi[ci])
        ntile = Tl + Th
        ps_agg = pA.tile([width, P], F32)
        kk = 0
        for stream, pos, Tc, NT, d_tab, t_idx, gp, col_off in (
            ("lo", lo_pos, Tl, NT_lo, d_lotab, t_idxlo, gplo, 0),
            ("hi", hi_pos, Th, NT_hi, d_hitab, t_idxhi, gphi, NT_lo),
        ):
            st = state[stream]
            for k in range(Tc):
                t = pos + k
                if t == st[0]:
                    nb = min(GK, NT - t)
                    g_tile = gp.tile([P, GK * width], F32, tag="g" + stream)
                    nc.gpsimd.dma_gather(
                        out_ap=g_tile[:, : nb * width].rearrange(
                            "p (t c) -> p t c", c=width),
                        in_ap=d_tab.ap()[:, :],
                        idxs_ap=t_idx[:, t * 8 : (t + nb) * 8],
                        num_idxs=nb * P,
                        num_idxs_reg=nb * P,
                        elem_size=width,
                    )
                    st[0], st[1], st[2] = t + nb, g_tile, t
                s_t = sp.tile([P, P], F32, tag="s")
                nc.vector.tensor_scalar(
                    out=s_t[:],
                    in0=t_iota[:],
                    scalar1=t_ids[:, col_off + t : col_off + t + 1],
                    scalar2=t_ivd[:, col_off + t : col_off + t + 1],
                    op0=OP.is_equal,
                    op1=OP.mult,
                )
                off = (t - st[2]) * width
                nc.tensor.matmul(
                    out=ps_agg[:],
                    lhsT=st[1][:, off : off + width],
                    rhs=s_t[:],
                    start=(kk == 0),
                    stop=(close_group and kk == ntile - 1),
                )
                kk += 1
        lo_pos += Tl
        hi_pos += Th
        tail(ci, ps_agg)


def build_k1(cfg, T_lo, T_hi, compile=True):
    """Layer 1: x -> hT_own [H, NPC], hw2l_own [NPC, O] (row-major)."""
    NT_lo, NT_hi = int(T_lo.sum()), int(T_hi.sum())
    NTT = NT_lo + NT_hi
    C, H, O = cfg.C, cfg.H, cfg.O
    nc = _mk_nc()
    d_xlo = nc.dram_tensor("x_lo", (cfg.NLO, C), F32, kind="ExternalInput")
    d_xhi = nc.dram_tensor("x_hi", (cfg.NHI, C), F32, kind="ExternalInput")
    d_xT = nc.dram_tensor("xT_own", (C, cfg.NPC), F32, kind="ExternalInput")
    d_ilo = nc.dram_tensor("idxlo", (P, max(NT_lo, 1) * 8), I16, kind="ExternalInput")
    d_ihi = nc.dram_tensor("idxhi", (P, max(NT_hi, 1) * 8), I16, kind="ExternalInput")
    d_ids = nc.dram_tensor("ids", (P, NTT), F32, kind="ExternalInput")
    d_ivd = nc.dram_tensor("ivd", (P, NTT), F32, kind="ExternalInput")
    d_iota = nc.dram_tensor("iota", (P, P), F32, kind="ExternalInput")
    d_iden = nc.dram_tensor("iden", (P, P), F32, kind="ExternalInput")
    d_w1l = nc.dram_tensor("w1l", (C, H), F32, kind="ExternalInput")
    d_w1r = nc.dram_tensor("w1r", (C, H), F32, kind="ExternalInput")
    d_c1 = nc.dram_tensor("c1", (H, 1), F32, kind="ExternalInput")
    d_w2l = nc.dram_tensor("w2l", (H, O), F32, kind="ExternalInput")
    d_hT = nc.dram_tensor("hT", (H, cfg.NPC), F32, kind="ExternalOutput")
    d_hw2l = nc.dram_tensor("hw2l", (cfg.NPC, O), F32, kind="ExternalOutput")

    with tile.TileContext(nc) as tc:
        with (
            tc.tile_pool(name="const", bufs=1) as cp,
            tc.tile_pool(name="glo", bufs=3) as gplo,
            tc.tile_pool(name="ghi", bufs=3) as gphi,
            tc.tile_pool(name="sel", bufs=6) as sp,
            tc.tile_pool(name="work", bufs=3) as wp,
            tc.tile_pool(name="psA", bufs=2, space="PSUM") as pA,
            tc.tile_pool(name="psB", bufs=2, space="PSUM") as pB,
            tc.tile_pool(name="psC", bufs=2, space="PSUM") as pC,
            tc.tile_pool(name="psD", bufs=2, space="PSUM") as pD,
        ):
            def cload(name, d, shape, dt=F32):
                t = cp.tile(shape, dt, tag=name)
                nc.sync.dma_start(t[:], d.ap()[:, :])
                return t

            t_iota = cload("iota", d_iota, [P, P])
            t_iden = cload("iden", d_iden, [P, P])
            t_w1l = cload("w1l", d_w1l, [C, H])
            t_w1r = cload("w1r", d_w1r, [C, H])
            t_c1 = cload("c1", d_c1, [H, 1])
            t_w2l = cload("w2l", d_w2l, [H, O])
            t_ilo = cload("ilo", d_ilo, [P, max(NT_lo, 1) * 8], I16)
            t_ihi = cload("ihi", d_ihi, [P, max(NT_hi, 1) * 8], I16)
            t_ids = cload("ids", d_ids, [P, NTT])
            t_ivd = cload("ivd", d_ivd, [P, NTT])

            def tail(ci, ps_agg):
                agg_sb = wp.tile([P, P], F32, tag="agg")
                nc.scalar.copy(out=agg_sb[:], in_=ps_agg[:])
                xT_sb = wp.tile([P, P], F32, tag="xT")
                nc.sync.dma_start(xT_sb[:], d_xT.ap()[:, ci * P : (ci + 1) * P])
                ps_h = pB.tile([P, P], F32)
                nc.tensor.matmul(out=ps_h[:], lhsT=t_w1l[:], rhs=agg_sb[:],
                                 start=True, stop=False)
                nc.tensor.matmul(out=ps_h[:], lhsT=t_w1r[:], rhs=xT_sb[:],
                                 start=False, stop=True)
                hT_sb = wp.tile([P, P], F32, tag="hT")
                nc.scalar.activation(out=hT_sb[:], in_=ps_h[:], func=AF.Relu,
                                     bias=t_c1[:, :1], scale=1.0)
                nc.sync.dma_start(d_hT.ap()[:, ci * P : (ci + 1) * P], hT_sb[:])

                ps_w = pC.tile([O, P], F32)
                nc.tensor.matmul(out=ps_w[:], lhsT=t_w2l[:], rhs=hT_sb[:],
                                 start=True, stop=True)
                wT_sb = wp.tile([O, P], F32, tag="wT")
                nc.scalar.copy(out=wT_sb[:], in_=ps_w[:])
                ps_r = pD.tile([P, O], F32)
                nc.tensor.transpose(out=ps_r[:], in_=wT_sb[:],
                                    identity=t_iden[:O, :O])
                rm_sb = wp.tile([P, O], F32, tag="rm")
                nc.scalar.copy(out=rm_sb[:], in_=ps_r[:])
                nc.sync.dma_start(d_hw2l.ap()[ci * P : (ci + 1) * P, :], rm_sb[:])

            _agg_chunks(nc, cfg, T_lo, T_hi, d_xlo, d_xhi, t_ilo, t_ihi,
                        t_ids, t_ivd, t_iota, gplo, gphi, sp, pA, C, tail)

    if compile:
        nc.compile()
    return nc


def build_k2(cfg, T_lo, T_hi, compile=True):
    """Layer 2: hw2l (lo/hi split) + hT_own -> out_own [NPC, O] row-major."""
    NT_lo, NT_hi = int(T_lo.sum()), int(T_hi.sum())
    NTT = NT_lo + NT_hi
    H, O = cfg.H, cfg.O
    nc = _mk_nc()
    d_hwlo = nc.dram_tensor("hw_lo", (cfg.NLO, O), F32, kind="ExternalInput")
    d_hwhi = nc.dram_tensor("hw_hi", (cfg.NHI, O), F32, kind="ExternalInput")
    d_hT = nc.dram_tensor("hT_own", (H, cfg.NPC), F32, kind="ExternalInput")
    d_ilo = nc.dram_tensor("idxlo", (P, max(NT_lo, 1) * 8), I16, kind="ExternalInput")
    d_ihi = nc.dram_tensor("idxhi", (P, max(NT_hi, 1) * 8), I16, kind="ExternalInput")
    d_ids = nc.dram_tensor("ids", (P, NTT), F32, kind="ExternalInput")
    d_ivd = nc.dram_tensor("ivd", (P, NTT), F32, kind="ExternalInput")
    d_iota = nc.dram_tensor("iota", (P, P), F32, kind="ExternalInput")
    d_iden = nc.dram_tensor("iden", (P, P), F32, kind="ExternalInput")
    d_w2r = nc.dram_tensor("w2r", (H, O), F32, kind="ExternalInput")
    d_b2 = nc.dram_tensor("b2", (O, 1), F32, kind="ExternalInput")
    d_out = nc.dram_tensor("out_own", (cfg.NPC, O), F32, kind="ExternalOutput")

    with tile.TileContext(nc) as tc:
        with (
            tc.tile_pool(name="const", bufs=1) as cp,
            tc.tile_pool(name="glo", bufs=3) as gplo,
            tc.tile_pool(name="ghi", bufs=3) as gphi,
            tc.tile_pool(name="sel", bufs=6) as sp,
            tc.tile_pool(name="work", bufs=3) as wp,
            tc.tile_pool(name="psA", bufs=2, space="PSUM") as pA,
            tc.tile_pool(name="psD", bufs=2, space="PSUM") as pD,
        ):
            def cload(name, d, shape, dt=F32):
                t = cp.tile(shape, dt, tag=name)
                nc.sync.dma_start(t[:], d.ap()[:, :])
                return t

            t_iota = cload("iota", d_iota, [P, P])
            t_iden = cload("iden", d_iden, [P, P])
            t_w2r = cload("w2r", d_w2r, [H, O])
            t_b2 = cload("b2", d_b2, [O, 1])
            t_ilo = cload("ilo", d_ilo, [P, max(NT_lo, 1) * 8], I16)
            t_ihi = cload("ihi", d_ihi, [P, max(NT_hi, 1) * 8], I16)
            t_ids = cload("ids", d_ids, [P, NTT])
            t_ivd = cload("ivd", d_ivd, [P, NTT])
            t_hT = cp.tile([H, cfg.NPC], F32, tag="hT_all")
            nc.sync.dma_start(t_hT[:], d_hT.ap()[:, :])

            def tail(ci, ps_agg):
                nc.tensor.matmul(
                    out=ps_agg[:],
                    lhsT=t_w2r[:],
                    rhs=t_hT[:, ci * P : (ci + 1) * P],
                    start=False,
                    stop=True,
                )
                oT_sb = wp.tile([O, P], F32, tag="oT")
                nc.scalar.activation(out=oT_sb[:], in_=ps_agg[:],
                                     func=AF.Identity, bias=t_b2[:, :1], scale=1.0)
                ps_r = pD.tile([P, O], F32)
                nc.tensor.transpose(out=ps_r[:], in_=oT_sb[:],
                                    identity=t_iden[:O, :O])
                rm_sb = wp.tile([P, O], F32, tag="rm")
                nc.scalar.copy(out=rm_sb[:], in_=ps_r[:])
                nc.sync.dma_start(d_out.ap()[ci * P : (ci + 1) * P, :], rm_sb[:])

            _agg_chunks(nc, cfg, T_lo, T_hi, d_hwlo, d_hwhi, t_ilo, t_ihi,
                        t_ids, t_ivd, t_iota, gplo, gphi, sp, pA, O, tail,
                        close_group=False)

    if compile:
        nc.compile()
    return nc


def make_inmaps1(cfg, idxlo, idxhi, ids, ivd, x, W1_l, W1_r, b1,
                 bn_gamma, bn_beta, bn_mean, bn_var, W2_l):
    s = (np.asarray(bn_gamma, np.float64)
         / np.sqrt(np.asarray(bn_var, np.float64) + 1e-5))
    w1l_f = (np.asarray(W1_l, np.float64) * s[None, :]).astype(np.float32)
    w1r_f = (np.asarray(W1_r, np.float64) * s[None, :]).astype(np.float32)
    c1 = ((np.asarray(b1, np.float64) - np.asarray(bn_mean, np.float64)) * s
          + np.asarray(bn_beta, np.float64)).astype(np.float32).reshape(cfg.H, 1)
    x_pad = np.zeros((cfg.NP, cfg.C), np.float32)
    x_pad[: cfg.N] = np.asarray(x, np.float32)
    x_lo = np.ascontiguousarray(x_pad[: cfg.NLO])
    x_hi = np.ascontiguousarray(x_pad[SPLIT : SPLIT + cfg.NHI]) \
        if cfg.NP > SPLIT else np.zeros((cfg.NHI, cfg.C), np.float32)
    iota = np.tile(np.arange(P, dtype=np.float32), (P, 1))
    iden = np.eye(P, dtype=np.float32)
    w2l = np.ascontiguousarray(np.asarray(W2_l, np.float32))
    maps = []
    for c in range(N_CORES):
        xT_own = np.ascontiguousarray(x_pad[c * cfg.NPC : (c + 1) * cfg.NPC].T)
        maps.append(dict(
            x_lo=x_lo, x_hi=x_hi, xT_own=xT_own,
            idxlo=idxlo[c], idxhi=idxhi[c], ids=ids[c], ivd=ivd[c],
            iota=iota, iden=iden,
            w1l=w1l_f, w1r=w1r_f, c1=c1, w2l=w2l,
        ))
    return maps


def make_inmaps2(cfg, idxlo, idxhi, ids, ivd, hw2l_full, hT_parts, W2_r, b2):
    iota = np.tile(np.arange(P, dtype=np.float32), (P, 1))
    iden = np.eye(P, dtype=np.float32)
    w2r = np.ascontiguousarray(np.asarray(W2_r, np.float32))
    b2c = np.asarray(b2, np.float32).reshape(cfg.O, 1)
    hw_lo = np.ascontiguousarray(hw2l_full[: cfg.NLO])
    hw_hi = np.ascontiguousarray(hw2l_full[SPLIT : SPLIT + cfg.NHI]) \
        if cfg.NP > SPLIT else np.zeros((cfg.NHI, cfg.O), np.float32)
    maps = []
    for c in range(N_CORES):
        maps.append(dict(
            hw_lo=hw_lo, hw_hi=hw_hi, hT_own=hT_parts[c],
            idxlo=idxlo[c], idxhi=idxhi[c], ids=ids[c], ivd=ivd[c],
            iota=iota, iden=iden, w2r=w2r, b2=b2c,
        ))
    return maps


_cache = {}


def _get_programs(T_lo, T_hi):
    key = (tuple(int(t) for t in T_lo), tuple(int(t) for t in T_hi))
    if key not in _cache:
        _cache[key] = (build_k1(CFG, T_lo, T_hi), build_k2(CFG, T_lo, T_hi))
    return _cache[key]


def kernel(x, edge_index, W1_l, W1_r, b1, bn_gamma, bn_beta, bn_mean, bn_var,
           W2_l, W2_r, b2, _results=None):
    cfg = CFG
    T_lo, T_hi, idxlo, idxhi, ids, ivd = _preprocess(edge_index, cfg)
    nc1, nc2 = _get_programs(T_lo, T_hi)
    maps1 = make_inmaps1(cfg, idxlo, idxhi, ids, ivd, x, W1_l, W1_r, b1,
                         bn_gamma, bn_beta, bn_mean, bn_var, W2_l)
    r1 = run_bass_kernel_spmd(nc1, maps1, list(range(N_CORES)))
    hw2l_full = np.concatenate(
        [r1.results[c]["hw2l"] for c in range(N_CORES)], axis=0)
    hT_parts = [r1.results[c]["hT"] for c in range(N_CORES)]
    maps2 = make_inmaps2(cfg, idxlo, idxhi, ids, ivd, hw2l_full, hT_parts,
                         W2_r, b2)
    r2 = run_bass_kernel_spmd(nc2, maps2, list(range(N_CORES)))
    out = np.concatenate(
        [r2.results[c]["out_own"] for c in range(N_CORES)], axis=0)
    if _results is not None:
        _results.extend([r1, r2])
    return out[: cfg.N]



# revision 4
# speedup vs baseline: 1.6307x; 1.6307x over previous
"""2-layer GraphSAGE (mean aggr + BN(eval) + ReLU) on Trainium2, 8-core SPMD.

Strategy (graph/data parallel, dst-node sharding), v2:
  - Host: sort edges by dst, partition dst nodes into 8 ranges (49 chunks of
    128 nodes per core). Within each chunk, split edges into lo (src < 32768)
    and hi (src >= 32768) streams (dma_gather indices are signed int16), pad
    each stream to 128-edge tiles with per-chunk tile counts shared across
    cores (SPMD: one program, 8 cores). Gather indices are pre-wrapped into
    the dma_gather layout (idx i at [16g + i%16, i//16], 8 group replicas).
  - All gather tables and matmul operands are bf16 (PSUM accumulates fp32);
    gather calls round-robin over 4 SWDGE queues so descriptor generation
    runs on all four Q7 core pairs concurrently (4x Pool throughput).
  - Device layer 1 (per core, per 128-node chunk):
      dma_gather x rows (lo/hi bf16 tables, GK tiles per call)
      build selection S[e, j] = (dstloc[e]==j) * invdeg[dst[e]] on DVE (bf16)
      PSUM aggT[ch, node] += G^T @ S  (PE, bf16)
      h^T = relu(W1l'^T @ aggT + W1r'^T @ xT + c1)   (BN folded into W/c)
      also emit hW2l = (h @ W2_l) row-major bf16 for layer-2 gathers
  - Host: allgather hW2l parts, zero-pad 64->128 cols, split at 32768.
  - Device layer 2: same aggregation over hW2l rows + W2_r self term + b2.
"""

import numpy as np
import ml_dtypes

import concourse.bacc as bacc
import concourse.bass as bass
import concourse.mybir as mybir
import concourse.tile as tile
from concourse.bass_utils import run_bass_kernel_spmd

F32 = mybir.dt.float32
BF16 = mybir.dt.bfloat16
I16 = mybir.dt.int16
AF = mybir.ActivationFunctionType
OP = mybir.AluOpType
BF = ml_dtypes.bfloat16

N_CORES = 8
P = 128
SPLIT = 32768                           # int16 signed index limit
NQ = 4                                  # SWDGE queues (4 Q7 core pairs)


class Cfg:
    def __init__(self, n_nodes, c_in, c_hid, c_out, cpc):
        self.N = n_nodes
        self.C, self.H, self.O = c_in, c_hid, c_out
        self.CPC = cpc                  # 128-node chunks per core
        self.NPC = cpc * P              # nodes per core
        self.NP = self.NPC * N_CORES    # padded node count
        assert self.NP >= n_nodes
        self.NLO = min(self.NP, SPLIT)          # rows in lo table
        self.NHI = max(self.NP - SPLIT, 1)      # rows in hi table


CFG = Cfg(50000, 128, 128, 64, 49)      # NP = 50176
GK = 8                                  # edge tiles per dma_gather call
                                        # (>1024 idxs per call crashes HW)


def _wrap_idx(a):
    """[NC, 128, NT] int32 -> dma_gather wrapped [NC, 128, NT*8] int16.
    Per tile t, edge e: wrapped[16g + e%16, t*8 + e//16] = a[e, t]."""
    ncr, _, nt = a.shape
    w16 = (a.reshape(ncr, 8, 16, nt).transpose(0, 2, 3, 1)
           .reshape(ncr, 16, nt * 8))
    return np.tile(w16, (1, 8, 1)).astype(np.uint16).view(np.int16)


def _preprocess(edge_index, cfg):
    """Sort/partition edges; split per chunk into lo/hi gather streams.
    Returns per-chunk tile counts (shared across cores), wrapped int16 index
    arrays, and [128, NT_lo+NT_hi] ids/ivd tile arrays per core."""
    src = np.asarray(edge_index[0]).astype(np.int64)
    dst = np.asarray(edge_index[1]).astype(np.int64)
    order = np.argsort(dst, kind="stable")
    s_src = src[order].astype(np.int32)
    s_dst = dst[order].astype(np.int32)
    deg = np.bincount(dst, minlength=cfg.NP).astype(np.float32)
    invdeg = (1.0 / np.maximum(deg, 1.0)).astype(np.float32)
    bounds = np.searchsorted(s_dst, np.arange(0, cfg.NP + 1, P)).astype(np.int64)
    islo = s_src < SPLIT

    nlo = np.zeros((N_CORES, cfg.CPC), np.int64)
    nhi = np.zeros((N_CORES, cfg.CPC), np.int64)
    for c in range(N_CORES):
        for ci in range(cfg.CPC):
            g = c * cfg.CPC + ci
            e0, e1 = int(bounds[g]), int(bounds[g + 1])
            nlo[c, ci] = int(islo[e0:e1].sum())
            nhi[c, ci] = (e1 - e0) - nlo[c, ci]
    T_lo = ((nlo.max(axis=0) + P - 1) // P).astype(np.int64)
    T_hi = ((nhi.max(axis=0) + P - 1) // P).astype(np.int64)
    T_lo[(T_lo == 0) & (T_hi == 0)] = 1     # every chunk needs >=1 matmul
    NT_lo, NT_hi = int(T_lo.sum()), int(T_hi.sum())
    lo_start = np.zeros(cfg.CPC, np.int64)
    lo_start[1:] = np.cumsum(T_lo)[:-1]
    hi_start = np.zeros(cfg.CPC, np.int64)
    hi_start[1:] = np.cumsum(T_hi)[:-1]
    NTT = NT_lo + NT_hi

    src_lo = np.zeros((N_CORES, P, max(NT_lo, 1)), np.int32)
    src_hi = np.zeros((N_CORES, P, max(NT_hi, 1)), np.int32)
    ids = np.full((N_CORES, P, NTT), 300.0, np.float32)
    ivd = np.zeros((N_CORES, P, NTT), np.float32)
    for c in range(N_CORES):
        for ci in range(cfg.CPC):
            g = c * cfg.CPC + ci
            e0, e1 = int(bounds[g]), int(bounds[g + 1])
            if e1 == e0:
                continue
            es = s_src[e0:e1]
            ed = s_dst[e0:e1]
            m = es < SPLIT
            for sel, st, s_off, arr, base in (
                (m, lo_start, 0, src_lo, 0),
                (~m, hi_start, NT_lo, src_hi, SPLIT),
            ):
                vs = es[sel] - base
                vd = ed[sel]
                n = len(vs)
                if n == 0:
                    continue
                j = np.arange(n)
                t = st[ci] + j // P
                pp = j % P
                arr[c, pp, t] = vs
                ids[c, pp, s_off + t] = (vd - (c * cfg.NPC + ci * P)).astype(np.float32)
                ivd[c, pp, s_off + t] = invdeg[vd]
    return T_lo, T_hi, _wrap_idx(src_lo), _wrap_idx(src_hi), ids, ivd


def _mk_nc():
    return bacc.Bacc(
        "TRN2",
        target_bir_lowering=False,
        debug=False,
        enable_asserts=False,
        num_devices=N_CORES,
        num_swdge_queues=NQ,
    )


def _agg_chunks(nc, cfg, T_lo, T_hi, d_lotab, d_hitab, t_idxlo, t_idxhi,
                t_ids, t_ivd, t_iota, gplo, gphi, sp, pA, width, lw, tail,
                close_group=True):
    """Shared aggregation loop: per chunk accumulate G^T @ S into PSUM over
    the lo then hi tile streams, then call tail(ci, ps_agg). Gather calls
    round-robin over the SWDGE queues. `width` is PSUM rows (contraction out),
    `lw` is valid lhsT cols per gathered 128-col tile."""
    NT_lo = int(T_lo.sum())
    NT_hi = int(T_hi.sum())
    state = {"lo": [0, None, 0], "hi": [0, None, 0]}  # cursor, tile, base
    qcnt = [0]
    lo_pos = hi_pos = 0
    for ci in range(cfg.CPC):
        Tl, Th = int(T_lo[ci]), int(T_hi[ci])
        ntile = Tl + Th
        ps_agg = pA.tile([width, P], F32)
        kk = 0
        for stream, pos, Tc, NT, d_tab, t_idx, gp, col_off in (
            ("lo", lo_pos, Tl, NT_lo, d_lotab, t_idxlo, gplo, 0),
            ("hi", hi_pos, Th, NT_hi, d_hitab, t_idxhi, gphi, NT_lo),
        ):
            st = state[stream]
            for k in range(Tc):
                t = pos + k
                if t == st[0]:
                    nb = min(GK, NT - t)
                    g_tile = gp.tile([P, GK * P], BF16, tag="g" + stream)
                    nc.gpsimd.dma_gather(
                        out_ap=g_tile[:, : nb * P].rearrange(
                            "p (t c) -> p t c", c=P),
                        in_ap=d_tab.ap()[:, :],
                        idxs_ap=t_idx[:, t * 8 : (t + nb) * 8],
                        num_idxs=nb * P,
                        num_idxs_reg=nb * P,
                        elem_size=P,
                        queue_num=qcnt[0] % NQ,
                    )
                    qcnt[0] += 1
                    st[0], st[1], st[2] = t + nb, g_tile, t
                s_t = sp.tile([P, P], BF16, tag="s")
                nc.vector.tensor_scalar(
                    out=s_t[:],
                    in0=t_iota[:],
                    scalar1=t_ids[:, col_off + t : col_off + t + 1],
                    scalar2=t_ivd[:, col_off + t : col_off + t + 1],
                    op0=OP.is_equal,
                    op1=OP.mult,
                )
                off = (t - st[2]) * P
                nc.tensor.matmul(
                    out=ps_agg[:],
                    lhsT=st[1][:, off : off + lw],
                    rhs=s_t[:],
                    start=(kk == 0),
                    stop=(close_group and kk == ntile - 1),
                )
                kk += 1
        lo_pos += Tl
        hi_pos += Th
        tail(ci, ps_agg)


def build_k1(cfg, T_lo, T_hi, compile=True):
    """Layer 1: x -> hT_own [H, NPC] bf16, hw2l_own [NPC, O] bf16 row-major."""
    NT_lo, NT_hi = int(T_lo.sum()), int(T_hi.sum())
    NTT = NT_lo + NT_hi
    C, H, O = cfg.C, cfg.H, cfg.O
    nc = _mk_nc()
    d_xlo = nc.dram_tensor("x_lo", (cfg.NLO, C), BF16, kind="ExternalInput")
    d_xhi = nc.dram_tensor("x_hi", (cfg.NHI, C), BF16, kind="ExternalInput")
    d_xT = nc.dram_tensor("xT_own", (C, cfg.NPC), BF16, kind="ExternalInput")
    d_ilo = nc.dram_tensor("idxlo", (P, max(NT_lo, 1) * 8), I16, kind="ExternalInput")
    d_ihi = nc.dram_tensor("idxhi", (P, max(NT_hi, 1) * 8), I16, kind="ExternalInput")
    d_ids = nc.dram_tensor("ids", (P, NTT), F32, kind="ExternalInput")
    d_ivd = nc.dram_tensor("ivd", (P, NTT), F32, kind="ExternalInput")
    d_iota = nc.dram_tensor("iota", (P, P), BF16, kind="ExternalInput")
    d_iden = nc.dram_tensor("iden", (P, P), BF16, kind="ExternalInput")
    d_w1l = nc.dram_tensor("w1l", (C, H), BF16, kind="ExternalInput")
    d_w1r = nc.dram_tensor("w1r", (C, H), BF16, kind="ExternalInput")
    d_c1 = nc.dram_tensor("c1", (H, 1), F32, kind="ExternalInput")
    d_w2l = nc.dram_tensor("w2l", (H, O), BF16, kind="ExternalInput")
    d_hT = nc.dram_tensor("hT", (H, cfg.NPC), BF16, kind="ExternalOutput")
    d_hw2l = nc.dram_tensor("hw2l", (cfg.NPC, O), BF16, kind="ExternalOutput")

    with tile.TileContext(nc) as tc:
        with (
            tc.tile_pool(name="const", bufs=1) as cp,
            tc.tile_pool(name="glo", bufs=8) as gplo,
            tc.tile_pool(name="ghi", bufs=8) as gphi,
            tc.tile_pool(name="sel", bufs=8) as sp,
            tc.tile_pool(name="work", bufs=3) as wp,
            tc.tile_pool(name="psA", bufs=2, space="PSUM") as pA,
            tc.tile_pool(name="psB", bufs=2, space="PSUM") as pB,
            tc.tile_pool(name="psC", bufs=2, space="PSUM") as pC,
            tc.tile_pool(name="psD", bufs=2, space="PSUM") as pD,
        ):
            def cload(name, d, shape, dt=F32):
                t = cp.tile(shape, dt, tag=name)
                nc.sync.dma_start(t[:], d.ap()[:, :])
                return t

            t_iota = cload("iota", d_iota, [P, P], BF16)
            t_iden = cload("iden", d_iden, [P, P], BF16)
            t_w1l = cload("w1l", d_w1l, [C, H], BF16)
            t_w1r = cload("w1r", d_w1r, [C, H], BF16)
            t_c1 = cload("c1", d_c1, [H, 1])
            t_w2l = cload("w2l", d_w2l, [H, O], BF16)
            t_ilo = cload("ilo", d_ilo, [P, max(NT_lo, 1) * 8], I16)
            t_ihi = cload("ihi", d_ihi, [P, max(NT_hi, 1) * 8], I16)
            t_ids = cload("ids", d_ids, [P, NTT])
            t_ivd = cload("ivd", d_ivd, [P, NTT])

            def tail(ci, ps_agg):
                agg_sb = wp.tile([P, P], BF16, tag="agg")
                nc.scalar.copy(out=agg_sb[:], in_=ps_agg[:])
                xT_sb = wp.tile([P, P], BF16, tag="xT")
                nc.sync.dma_start(xT_sb[:], d_xT.ap()[:, ci * P : (ci + 1) * P])
                ps_h = pB.tile([P, P], F32)
                nc.tensor.matmul(out=ps_h[:], lhsT=t_w1l[:], rhs=agg_sb[:],
                                 start=True, stop=False)
                nc.tensor.matmul(out=ps_h[:], lhsT=t_w1r[:], rhs=xT_sb[:],
                                 start=False, stop=True)
                hT_sb = wp.tile([P, P], BF16, tag="hT")
                nc.scalar.activation(out=hT_sb[:], in_=ps_h[:], func=AF.Relu,
                                     bias=t_c1[:, :1], scale=1.0)
                nc.sync.dma_start(d_hT.ap()[:, ci * P : (ci + 1) * P], hT_sb[:])

                ps_w = pC.tile([O, P], F32)
                nc.tensor.matmul(out=ps_w[:], lhsT=t_w2l[:], rhs=hT_sb[:],
                                 start=True, stop=True)
                wT_sb = wp.tile([O, P], BF16, tag="wT")
                nc.scalar.copy(out=wT_sb[:], in_=ps_w[:])
                ps_r = pD.tile([P, O], BF16)
                nc.tensor.transpose(out=ps_r[:], in_=wT_sb[:],
                                    identity=t_iden[:O, :O])
                rm_sb = wp.tile([P, O], BF16, tag="rm")
                nc.scalar.copy(out=rm_sb[:], in_=ps_r[:])
                nc.sync.dma_start(d_hw2l.ap()[ci * P : (ci + 1) * P, :], rm_sb[:])

            _agg_chunks(nc, cfg, T_lo, T_hi, d_xlo, d_xhi, t_ilo, t_ihi,
                        t_ids, t_ivd, t_iota, gplo, gphi, sp, pA, C, P, tail)

    if compile:
        nc.compile()
    return nc


def build_k2(cfg, T_lo, T_hi, compile=True):
    """Layer 2: hw2l (bf16, padded to 128 cols, lo/hi split) + hT_own ->
    out_own [NPC, O] f32 row-major."""
    NT_lo, NT_hi = int(T_lo.sum()), int(T_hi.sum())
    NTT = NT_lo + NT_hi
    H, O = cfg.H, cfg.O
    nc = _mk_nc()
    d_hwlo = nc.dram_tensor("hw_lo", (cfg.NLO, P), BF16, kind="ExternalInput")
    d_hwhi = nc.dram_tensor("hw_hi", (cfg.NHI, P), BF16, kind="ExternalInput")
    d_hT = nc.dram_tensor("hT_own", (H, cfg.NPC), BF16, kind="ExternalInput")
    d_ilo = nc.dram_tensor("idxlo", (P, max(NT_lo, 1) * 8), I16, kind="ExternalInput")
    d_ihi = nc.dram_tensor("idxhi", (P, max(NT_hi, 1) * 8), I16, kind="ExternalInput")
    d_ids = nc.dram_tensor("ids", (P, NTT), F32, kind="ExternalInput")
    d_ivd = nc.dram_tensor("ivd", (P, NTT), F32, kind="ExternalInput")
    d_iota = nc.dram_tensor("iota", (P, P), BF16, kind="ExternalInput")
    d_iden = nc.dram_tensor("iden", (P, P), BF16, kind="ExternalInput")
    d_w2r = nc.dram_tensor("w2r", (H, O), BF16, kind="ExternalInput")
    d_b2 = nc.dram_tensor("b2", (O, 1), F32, kind="ExternalInput")
    d_out = nc.dram_tensor("out_own", (cfg.NPC, O), F32, kind="ExternalOutput")

    with tile.TileContext(nc) as tc:
        with (
            tc.tile_pool(name="const", bufs=1) as cp,
            tc.tile_pool(name="glo", bufs=8) as gplo,
            tc.tile_pool(name="ghi", bufs=8) as gphi,
            tc.tile_pool(name="sel", bufs=8) as sp,
            tc.tile_pool(name="work", bufs=3) as wp,
            tc.tile_pool(name="psA", bufs=2, space="PSUM") as pA,
            tc.tile_pool(name="psD", bufs=2, space="PSUM") as pD,
        ):
            def cload(name, d, shape, dt=F32):
                t = cp.tile(shape, dt, tag=name)
                nc.sync.dma_start(t[:], d.ap()[:, :])
                return t

            t_iota = cload("iota", d_iota, [P, P], BF16)
            t_iden = cload("iden", d_iden, [P, P], BF16)
            t_w2r = cload("w2r", d_w2r, [H, O], BF16)
            t_b2 = cload("b2", d_b2, [O, 1])
            t_ilo = cload("ilo", d_ilo, [P, max(NT_lo, 1) * 8], I16)
            t_ihi = cload("ihi", d_ihi, [P, max(NT_hi, 1) * 8], I16)
            t_ids = cload("ids", d_ids, [P, NTT])
            t_ivd = cload("ivd", d_ivd, [P, NTT])
            t_hT = cp.tile([H, cfg.NPC], BF16, tag="hT_all")
            nc.sync.dma_start(t_hT[:], d_hT.ap()[:, :])

            def tail(ci, ps_agg):
                nc.tensor.matmul(
                    out=ps_agg[:],
                    lhsT=t_w2r[:],
                    rhs=t_hT[:, ci * P : (ci + 1) * P],
                    start=False,
                    stop=True,
                )
                oT_sb = wp.tile([O, P], BF16, tag="oT")
                nc.scalar.activation(out=oT_sb[:], in_=ps_agg[:],
                                     func=AF.Identity, bias=t_b2[:, :1], scale=1.0)
                ps_r = pD.tile([P, O], BF16)
                nc.tensor.transpose(out=ps_r[:], in_=oT_sb[:],
                                    identity=t_iden[:O, :O])
                rm_sb = wp.tile([P, O], F32, tag="rm")
                nc.scalar.copy(out=rm_sb[:], in_=ps_r[:])
                nc.sync.dma_start(d_out.ap()[ci * P : (ci + 1) * P, :], rm_sb[:])

            _agg_chunks(nc, cfg, T_lo, T_hi, d_hwlo, d_hwhi, t_ilo, t_ihi,
                        t_ids, t_ivd, t_iota, gplo, gphi, sp, pA, O, O, tail,
                        close_group=False)

    if compile:
        nc.compile()
    return nc


def make_inmaps1(cfg, idxlo, idxhi, ids, ivd, x, W1_l, W1_r, b1,
                 bn_gamma, bn_beta, bn_mean, bn_var, W2_l):
    s = (np.asarray(bn_gamma, np.float64)
         / np.sqrt(np.asarray(bn_var, np.float64) + 1e-5))
    w1l_f = (np.asarray(W1_l, np.float64) * s[None, :]).astype(BF)
    w1r_f = (np.asarray(W1_r, np.float64) * s[None, :]).astype(BF)
    c1 = ((np.asarray(b1, np.float64) - np.asarray(bn_mean, np.float64)) * s
          + np.asarray(bn_beta, np.float64)).astype(np.float32).reshape(cfg.H, 1)
    x_pad = np.zeros((cfg.NP, cfg.C), BF)
    x_pad[: cfg.N] = np.asarray(x, np.float32).astype(BF)
    x_lo = np.ascontiguousarray(x_pad[: cfg.NLO])
    x_hi = np.ascontiguousarray(x_pad[SPLIT : SPLIT + cfg.NHI]) \
        if cfg.NP > SPLIT else np.zeros((cfg.NHI, cfg.C), BF)
    iota = np.tile(np.arange(P, dtype=np.float32), (P, 1)).astype(BF)
    iden = np.eye(P, dtype=np.float32).astype(BF)
    w2l = np.ascontiguousarray(np.asarray(W2_l, np.float32).astype(BF))
    maps = []
    for c in range(N_CORES):
        xT_own = np.ascontiguousarray(x_pad[c * cfg.NPC : (c + 1) * cfg.NPC].T)
        maps.append(dict(
            x_lo=x_lo, x_hi=x_hi, xT_own=xT_own,
            idxlo=idxlo[c], idxhi=idxhi[c], ids=ids[c], ivd=ivd[c],
            iota=iota, iden=iden,
            w1l=w1l_f, w1r=w1r_f, c1=c1, w2l=w2l,
        ))
    return maps


def make_inmaps2(cfg, idxlo, idxhi, ids, ivd, hw2l_full, hT_parts, W2_r, b2):
    iota = np.tile(np.arange(P, dtype=np.float32), (P, 1)).astype(BF)
    iden = np.eye(P, dtype=np.float32).astype(BF)
    w2r = np.ascontiguousarray(np.asarray(W2_r, np.float32).astype(BF))
    b2c = np.asarray(b2, np.float32).reshape(cfg.O, 1)
    hw_pad = np.zeros((cfg.NP, P), BF)
    hw_pad[:, : cfg.O] = hw2l_full
    hw_lo = np.ascontiguousarray(hw_pad[: cfg.NLO])
    hw_hi = np.ascontiguousarray(hw_pad[SPLIT : SPLIT + cfg.NHI]) \
        if cfg.NP > SPLIT else np.zeros((cfg.NHI, P), BF)
    maps = []
    for c in range(N_CORES):
        maps.append(dict(
            hw_lo=hw_lo, hw_hi=hw_hi, hT_own=hT_parts[c],
            idxlo=idxlo[c], idxhi=idxhi[c], ids=ids[c], ivd=ivd[c],
            iota=iota, iden=iden, w2r=w2r, b2=b2c,
        ))
    return maps


_cache = {}


def _get_programs(T_lo, T_hi):
    key = (tuple(int(t) for t in T_lo), tuple(int(t) for t in T_hi))
    if key not in _cache:
        _cache[key] = (build_k1(CFG, T_lo, T_hi), build_k2(CFG, T_lo, T_hi))
    return _cache[key]


def kernel(x, edge_index, W1_l, W1_r, b1, bn_gamma, bn_beta, bn_mean, bn_var,
           W2_l, W2_r, b2, _results=None):
    cfg = CFG
    T_lo, T_hi, idxlo, idxhi, ids, ivd = _preprocess(edge_index, cfg)
    nc1, nc2 = _get_programs(T_lo, T_hi)
    maps1 = make_inmaps1(cfg, idxlo, idxhi, ids, ivd, x, W1_l, W1_r, b1,
                         bn_gamma, bn_beta, bn_mean, bn_var, W2_l)
    r1 = run_bass_kernel_spmd(nc1, maps1, list(range(N_CORES)))
    hw2l_full = np.concatenate(
        [r1.results[c]["hw2l"] for c in range(N_CORES)], axis=0)
    hT_parts = [r1.results[c]["hT"] for c in range(N_CORES)]
    maps2 = make_inmaps2(cfg, idxlo, idxhi, ids, ivd, hw2l_full, hT_parts,
                         W2_r, b2)
    r2 = run_bass_kernel_spmd(nc2, maps2, list(range(N_CORES)))
    out = np.concatenate(
        [r2.results[c]["out_own"] for c in range(N_CORES)], axis=0)
    if _results is not None:
        _results.extend([r1, r2])
    return out[: cfg.N]


# revision 8
# speedup vs baseline: 2.2759x; 1.3956x over previous
"""2-layer GraphSAGE (mean aggr + BN(eval) + ReLU) on Trainium2, 8-core SPMD.

Strategy (graph/data parallel, dst-node sharding), v3:
  - Host: sort edges by dst, partition dst nodes into 8 ranges (49 chunks of
    128 nodes per core). Within each chunk, split edges into lo (src < 32768)
    and hi (src >= 32768) streams (dma_gather indices are signed int16), pad
    each stream to 128-edge tiles with per-chunk tile counts shared across
    cores (SPMD: one program, 8 cores). Gather indices are pre-wrapped into
    the dma_gather layout (idx i at [16g + i%16, i//16], 8 group replicas);
    streams are padded to GK-tile multiples with -1 indices (the SWDGE ucode
    strips trailing negatives for free).
  - All gather tables and matmul operands are bf16 (PSUM accumulates fp32).
  - Gather calls round-robin over 4 SWDGE queues so descriptor generation
    runs on all four Q7 core pairs concurrently (4x Pool throughput); a
    single hoisted num_idxs register avoids per-call RegisterMove WAR
    serialization on the Pool sequencer.
  - Selection tiles S[e, d] = (dst_local[e]==d) * invdeg[dst[e]] are built on
    the HOST in a partition-major layout sall[e, t*128+d] and streamed by
    plain DMA (2KB/partition/call) — no DVE work in the main loop.
  - Device layer 1 (per core, per 128-node chunk):
      PSUM aggT[ch, node] += G^T @ S  (PE, bf16; G dma_gathered per tile)
      h^T = relu(W1l'^T @ aggT + W1r'^T @ xT + c1)   (BN folded into W/c)
      also emit hW2l = (h @ W2_l) row-major bf16 for layer-2 gathers
  - Host: allgather hW2l parts, zero-pad 64->128 cols, split at 32768.
  - Device layer 2: same aggregation over hW2l rows + W2_r self term + b2.
"""

import numpy as np
import ml_dtypes

import concourse.bacc as bacc
import concourse.bass as bass
import concourse.mybir as mybir
import concourse.tile as tile
from concourse.bass_utils import run_bass_kernel_spmd

F32 = mybir.dt.float32
BF16 = mybir.dt.bfloat16
I16 = mybir.dt.int16
AF = mybir.ActivationFunctionType
OP = mybir.AluOpType
BF = ml_dtypes.bfloat16

N_CORES = 8
P = 128
SPLIT = 32768                           # int16 signed index limit
NQ = 4                                  # SWDGE queues (4 Q7 core pairs)


class Cfg:
    def __init__(self, n_nodes, c_in, c_hid, c_out, cpc):
        self.N = n_nodes
        self.C, self.H, self.O = c_in, c_hid, c_out
        self.CPC = cpc                  # 128-node chunks per core
        self.NPC = cpc * P              # nodes per core
        self.NP = self.NPC * N_CORES    # padded node count
        assert self.NP >= n_nodes
        self.NLO = min(self.NP, SPLIT)          # rows in lo table
        self.NHI = max(self.NP - SPLIT, 1)      # rows in hi table


CFG = Cfg(50000, 128, 128, 64, 49)      # NP = 50176
GK = 8                                  # edge tiles per dma_gather call
                                        # (>1024 idxs per call crashes HW)


def _wrap_idx(a):
    """[NC, 128, NT] int32 -> dma_gather wrapped [NC, 128, NT*8] int16.
    Per tile t, edge e: wrapped[16g + e%16, t*8 + e//16] = a[e, t]."""
    ncr, _, nt = a.shape
    w16 = (a.reshape(ncr, 8, 16, nt).transpose(0, 2, 3, 1)
           .reshape(ncr, 16, nt * 8))
    return np.tile(w16, (1, 8, 1)).astype(np.uint16).view(np.int16)


def _pad8(n):
    return (n + GK - 1) // GK * GK


def _preprocess(edge_index, cfg):
    """Sort/partition edges; split per chunk into lo/hi gather streams.
    Returns per-chunk tile counts (shared across cores), wrapped int16 index
    arrays (padded to GK-tile multiples with -1), and the host-built
    partition-major selection table sall [NC, 128, NTT_pad*128] bf16."""
    src = np.asarray(edge_index[0]).astype(np.int64)
    dst = np.asarray(edge_index[1]).astype(np.int64)
    order = np.argsort(dst, kind="stable")
    s_src = src[order].astype(np.int32)
    s_dst = dst[order].astype(np.int32)
    deg = np.bincount(dst, minlength=cfg.NP).astype(np.float32)
    invdeg = (1.0 / np.maximum(deg, 1.0)).astype(np.float32)
    bounds = np.searchsorted(s_dst, np.arange(0, cfg.NP + 1, P)).astype(np.int64)
    islo = s_src < SPLIT

    nlo = np.zeros((N_CORES, cfg.CPC), np.int64)
    nhi = np.zeros((N_CORES, cfg.CPC), np.int64)
    for c in range(N_CORES):
        for ci in range(cfg.CPC):
            g = c * cfg.CPC + ci
            e0, e1 = int(bounds[g]), int(bounds[g + 1])
            nlo[c, ci] = int(islo[e0:e1].sum())
            nhi[c, ci] = (e1 - e0) - nlo[c, ci]
    T_lo = ((nlo.max(axis=0) + P - 1) // P).astype(np.int64)
    T_hi = ((nhi.max(axis=0) + P - 1) // P).astype(np.int64)
    T_lo[(T_lo == 0) & (T_hi == 0)] = 1     # every chunk needs >=1 matmul
    NT_lo, NT_hi = int(T_lo.sum()), int(T_hi.sum())
    NT_lo_p, NT_hi_p = _pad8(NT_lo), _pad8(NT_hi)
    lo_start = np.zeros(cfg.CPC, np.int64)
    lo_start[1:] = np.cumsum(T_lo)[:-1]
    hi_start = np.zeros(cfg.CPC, np.int64)
    hi_start[1:] = np.cumsum(T_hi)[:-1]
    NTT = NT_lo_p + NT_hi_p

    src_lo = np.zeros((N_CORES, P, NT_lo_p), np.int32)
    src_hi = np.zeros((N_CORES, P, NT_hi_p), np.int32)
    ids = np.full((N_CORES, P, NTT), -1, np.int32)
    ivd = np.zeros((N_CORES, P, NTT), np.float32)
    for c in range(N_CORES):
        for ci in range(cfg.CPC):
            g = c * cfg.CPC + ci
            e0, e1 = int(bounds[g]), int(bounds[g + 1])
            if e1 == e0:
                continue
            es = s_src[e0:e1]
            ed = s_dst[e0:e1]
            m = es < SPLIT
            for sel, st, s_off, arr, base in (
                (m, lo_start, 0, src_lo, 0),
                (~m, hi_start, NT_lo_p, src_hi, SPLIT),
            ):
                vs = es[sel] - base
                vd = ed[sel]
                n = len(vs)
                if n == 0:
                    continue
                j = np.arange(n)
                t = st[ci] + j // P
                pp = j % P
                arr[c, pp, t] = vs
                ids[c, pp, s_off + t] = vd - (c * cfg.NPC + ci * P)
                ivd[c, pp, s_off + t] = invdeg[vd]
    # partition-major selection table: sall[c, e, t*128 + d]
    sall = np.zeros((N_CORES, P, NTT * P), BF)
    for c in range(N_CORES):
        ee, tt = np.nonzero(ids[c] >= 0)
        dd = ids[c, ee, tt]
        sall[c, ee, tt * P + dd] = ivd[c, ee, tt]
    return T_lo, T_hi, _wrap_idx(src_lo), _wrap_idx(src_hi), sall


def _mk_nc():
    return bacc.Bacc(
        "TRN2",
        target_bir_lowering=False,
        debug=False,
        enable_asserts=False,
        num_devices=N_CORES,
        num_swdge_queues=NQ,
    )


def _agg_chunks(nc, cfg, T_lo, T_hi, d_lotab, d_hitab, t_idxlo, t_idxhi,
                d_sall, reg_n, gplo, gphi, sp, pA, width, lw, tail,
                close_group=True):
    """Shared aggregation loop: per chunk accumulate G^T @ S into PSUM over
    the lo then hi tile streams, then call tail(ci, ps_agg). Gather calls
    round-robin over the SWDGE queues; S tiles arrive by plain DMA in GK
    groups. `width` is PSUM rows, `lw` is valid lhsT cols per 128-col tile."""
    NT_lo = int(T_lo.sum())
    NT_hi = int(T_hi.sum())
    NT_lo_p, NT_hi_p = _pad8(NT_lo), _pad8(NT_hi)
    # cursor, gather tile, base, sel tile
    state = {"lo": [0, None, 0, None], "hi": [0, None, 0, None]}
    qcnt = [0]
    lo_pos = hi_pos = 0
    for ci in range(cfg.CPC):
        Tl, Th = int(T_lo[ci]), int(T_hi[ci])
        ntile = Tl + Th
        ps_agg = pA.tile([width, P], F32)
        kk = 0
        for stream, pos, Tc, NTp, d_tab, t_idx, gp, col_off in (
            ("lo", lo_pos, Tl, NT_lo_p, d_lotab, t_idxlo, gplo, 0),
            ("hi", hi_pos, Th, NT_hi_p, d_hitab, t_idxhi, gphi, NT_lo_p),
        ):
            st = state[stream]
            for k in range(Tc):
                t = pos + k
                if t == st[0]:
                    g_tile = gp.tile([P, GK * P], BF16, tag="g" + stream)
                    nc.gpsimd.dma_gather(
                        out_ap=g_tile[:].rearrange("p (t c) -> p t c", c=P),
                        in_ap=d_tab.ap()[:, :],
                        idxs_ap=t_idx[:, t * 8 : (t + GK) * 8],
                        num_idxs=GK * P,
                        num_idxs_reg=GK * P,
                        elem_size=P,
                        queue_num=qcnt[0] % NQ,
                    )
                    qcnt[0] += 1
                    s_grp = sp.tile([P, GK * P], BF16, tag="s" + stream)
                    nc.sync.dma_start(
                        s_grp[:],
                        d_sall.ap()[:, (col_off + t) * P : (col_off + t + GK) * P])
                    st[0], st[1], st[2], st[3] = t + GK, g_tile, t, s_grp
                off = (t - st[2]) * P
                nc.tensor.matmul(
                    out=ps_agg[:],
                    lhsT=st[1][:, off : off + lw],
                    rhs=st[3][:, off : off + P],
                    start=(kk == 0),
                    stop=(close_group and kk == ntile - 1),
                )
                kk += 1
        lo_pos += Tl
        hi_pos += Th
        tail(ci, ps_agg)


def build_k1(cfg, T_lo, T_hi, compile=True):
    """Layer 1: x -> hT_own [H, NPC] bf16, hw2l_own [NPC, O] bf16 row-major."""
    NT_lo_p = _pad8(int(T_lo.sum()))
    NT_hi_p = _pad8(int(T_hi.sum()))
    NTT = NT_lo_p + NT_hi_p
    C, H, O = cfg.C, cfg.H, cfg.O
    nc = _mk_nc()
    d_xlo = nc.dram_tensor("x_lo", (cfg.NLO, C), BF16, kind="ExternalInput")
    d_xhi = nc.dram_tensor("x_hi", (cfg.NHI, C), BF16, kind="ExternalInput")
    d_xT = nc.dram_tensor("xT_own", (C, cfg.NPC), BF16, kind="ExternalInput")
    d_ilo = nc.dram_tensor("idxlo", (P, NT_lo_p * 8), I16, kind="ExternalInput")
    d_ihi = nc.dram_tensor("idxhi", (P, NT_hi_p * 8), I16, kind="ExternalInput")
    d_sall = nc.dram_tensor("sall", (P, NTT * P), BF16, kind="ExternalInput")
    d_iden = nc.dram_tensor("iden", (P, P), BF16, kind="ExternalInput")
    d_w1l = nc.dram_tensor("w1l", (C, H), BF16, kind="ExternalInput")
    d_w1r = nc.dram_tensor("w1r", (C, H), BF16, kind="ExternalInput")
    d_c1 = nc.dram_tensor("c1", (H, 1), F32, kind="ExternalInput")
    d_w2l = nc.dram_tensor("w2l", (H, O), BF16, kind="ExternalInput")
    d_hT = nc.dram_tensor("hT", (H, cfg.NPC), BF16, kind="ExternalOutput")
    d_hw2l = nc.dram_tensor("hw2l", (cfg.NPC, O), BF16, kind="ExternalOutput")

    with tile.TileContext(nc) as tc:
        with (
            tc.tile_pool(name="const", bufs=1) as cp,
            tc.tile_pool(name="glo", bufs=6) as gplo,
            tc.tile_pool(name="ghi", bufs=6) as gphi,
            tc.tile_pool(name="sel", bufs=6) as sp,
            tc.tile_pool(name="work", bufs=4) as wp,
            tc.tile_pool(name="psA", bufs=2, space="PSUM") as pA,
            tc.tile_pool(name="psB", bufs=2, space="PSUM") as pB,
            tc.tile_pool(name="psC", bufs=2, space="PSUM") as pC,
            tc.tile_pool(name="psD", bufs=2, space="PSUM") as pD,
        ):
            def cload(name, d, shape, dt=F32):
                t = cp.tile(shape, dt, tag=name)
                nc.sync.dma_start(t[:], d.ap()[:, :])
                return t

            t_iden = cload("iden", d_iden, [P, P], BF16)
            t_w1l = cload("w1l", d_w1l, [C, H], BF16)
            t_w1r = cload("w1r", d_w1r, [C, H], BF16)
            t_c1 = cload("c1", d_c1, [H, 1])
            t_w2l = cload("w2l", d_w2l, [H, O], BF16)
            t_ilo = cload("ilo", d_ilo, [P, NT_lo_p * 8], I16)
            t_ihi = cload("ihi", d_ihi, [P, NT_hi_p * 8], I16)
            reg_n = None

            def tail(ci, ps_agg):
                agg_sb = wp.tile([P, P], BF16, tag="agg")
                nc.scalar.copy(out=agg_sb[:], in_=ps_agg[:])
                xT_sb = wp.tile([P, P], BF16, tag="xT")
                nc.sync.dma_start(xT_sb[:], d_xT.ap()[:, ci * P : (ci + 1) * P])
                ps_h = pB.tile([P, P], F32)
                nc.tensor.matmul(out=ps_h[:], lhsT=t_w1l[:], rhs=agg_sb[:],
                                 start=True, stop=False)
                nc.tensor.matmul(out=ps_h[:], lhsT=t_w1r[:], rhs=xT_sb[:],
                                 start=False, stop=True)
                hT_sb = wp.tile([P, P], BF16, tag="hT")
                nc.scalar.activation(out=hT_sb[:], in_=ps_h[:], func=AF.Relu,
                                     bias=t_c1[:, :1], scale=1.0)
                nc.sync.dma_start(d_hT.ap()[:, ci * P : (ci + 1) * P], hT_sb[:])

                ps_w = pC.tile([O, P], F32)
                nc.tensor.matmul(out=ps_w[:], lhsT=t_w2l[:], rhs=hT_sb[:],
                                 start=True, stop=True)
                wT_sb = wp.tile([O, P], BF16, tag="wT")
                nc.scalar.copy(out=wT_sb[:], in_=ps_w[:])
                ps_r = pD.tile([P, O], BF16)
                nc.tensor.transpose(out=ps_r[:], in_=wT_sb[:],
                                    identity=t_iden[:O, :O])
                rm_sb = wp.tile([P, O], BF16, tag="rm")
                nc.scalar.copy(out=rm_sb[:], in_=ps_r[:])
                nc.sync.dma_start(d_hw2l.ap()[ci * P : (ci + 1) * P, :], rm_sb[:])

            _agg_chunks(nc, cfg, T_lo, T_hi, d_xlo, d_xhi, t_ilo, t_ihi,
                        d_sall, reg_n, gplo, gphi, sp, pA, C, P, tail)

    if compile:
        nc.compile()
    return nc


def build_k2(cfg, T_lo, T_hi, compile=True):
    """Layer 2: hw2l (bf16, padded to 128 cols, lo/hi split) + hT_own ->
    out_own [NPC, O] f32 row-major."""
    NT_lo_p = _pad8(int(T_lo.sum()))
    NT_hi_p = _pad8(int(T_hi.sum()))
    NTT = NT_lo_p + NT_hi_p
    H, O = cfg.H, cfg.O
    nc = _mk_nc()
    d_hwlo = nc.dram_tensor("hw_lo", (cfg.NLO, P), BF16, kind="ExternalInput")
    d_hwhi = nc.dram_tensor("hw_hi", (cfg.NHI, P), BF16, kind="ExternalInput")
    d_hT = nc.dram_tensor("hT_own", (H, cfg.NPC), BF16, kind="ExternalInput")
    d_ilo = nc.dram_tensor("idxlo", (P, NT_lo_p * 8), I16, kind="ExternalInput")
    d_ihi = nc.dram_tensor("idxhi", (P, NT_hi_p * 8), I16, kind="ExternalInput")
    d_sall = nc.dram_tensor("sall", (P, NTT * P), BF16, kind="ExternalInput")
    d_iden = nc.dram_tensor("iden", (P, P), BF16, kind="ExternalInput")
    d_w2r = nc.dram_tensor("w2r", (H, O), BF16, kind="ExternalInput")
    d_b2 = nc.dram_tensor("b2", (O, 1), F32, kind="ExternalInput")
    d_out = nc.dram_tensor("out_own", (cfg.NPC, O), F32, kind="ExternalOutput")

    with tile.TileContext(nc) as tc:
        with (
            tc.tile_pool(name="const", bufs=1) as cp,
            tc.tile_pool(name="glo", bufs=6) as gplo,
            tc.tile_pool(name="ghi", bufs=6) as gphi,
            tc.tile_pool(name="sel", bufs=6) as sp,
            tc.tile_pool(name="work", bufs=4) as wp,
            tc.tile_pool(name="psA", bufs=3, space="PSUM") as pA,
            tc.tile_pool(name="psD", bufs=2, space="PSUM") as pD,
        ):
            def cload(name, d, shape, dt=F32):
                t = cp.tile(shape, dt, tag=name)
                nc.sync.dma_start(t[:], d.ap()[:, :])
                return t

            t_iden = cload("iden", d_iden, [P, P], BF16)
            t_w2r = cload("w2r", d_w2r, [H, O], BF16)
            t_b2 = cload("b2", d_b2, [O, 1])
            t_ilo = cload("ilo", d_ilo, [P, NT_lo_p * 8], I16)
            t_ihi = cload("ihi", d_ihi, [P, NT_hi_p * 8], I16)
            t_hT = cp.tile([H, cfg.NPC], BF16, tag="hT_all")
            nc.sync.dma_start(t_hT[:], d_hT.ap()[:, :])
            reg_n = None

            def tail(ci, ps_agg):
                nc.tensor.matmul(
                    out=ps_agg[:],
                    lhsT=t_w2r[:],
                    rhs=t_hT[:, ci * P : (ci + 1) * P],
                    start=False,
                    stop=True,
                )
                oT_sb = wp.tile([O, P], BF16, tag="oT")
                nc.scalar.activation(out=oT_sb[:], in_=ps_agg[:],
                                     func=AF.Identity, bias=t_b2[:, :1], scale=1.0)
                ps_r = pD.tile([P, O], BF16)
                nc.tensor.transpose(out=ps_r[:], in_=oT_sb[:],
                                    identity=t_iden[:O, :O])
                rm_sb = wp.tile([P, O], F32, tag="rm")
                nc.scalar.copy(out=rm_sb[:], in_=ps_r[:])
                nc.sync.dma_start(d_out.ap()[ci * P : (ci + 1) * P, :], rm_sb[:])

            _agg_chunks(nc, cfg, T_lo, T_hi, d_hwlo, d_hwhi, t_ilo, t_ihi,
                        d_sall, reg_n, gplo, gphi, sp, pA, O, O, tail,
                        close_group=False)

    if compile:
        nc.compile()
    return nc


def make_inmaps1(cfg, idxlo, idxhi, sall, x, W1_l, W1_r, b1,
                 bn_gamma, bn_beta, bn_mean, bn_var, W2_l):
    s = (np.asarray(bn_gamma, np.float64)
         / np.sqrt(np.asarray(bn_var, np.float64) + 1e-5))
    w1l_f = (np.asarray(W1_l, np.float64) * s[None, :]).astype(BF)
    w1r_f = (np.asarray(W1_r, np.float64) * s[None, :]).astype(BF)
    c1 = ((np.asarray(b1, np.float64) - np.asarray(bn_mean, np.float64)) * s
          + np.asarray(bn_beta, np.float64)).astype(np.float32).reshape(cfg.H, 1)
    x_pad = np.zeros((cfg.NP, cfg.C), BF)
    x_pad[: cfg.N] = np.asarray(x, np.float32).astype(BF)
    x_lo = np.ascontiguousarray(x_pad[: cfg.NLO])
    x_hi = np.ascontiguousarray(x_pad[SPLIT : SPLIT + cfg.NHI]) \
        if cfg.NP > SPLIT else np.zeros((cfg.NHI, cfg.C), BF)
    iden = np.eye(P, dtype=np.float32).astype(BF)
    w2l = np.ascontiguousarray(np.asarray(W2_l, np.float32).astype(BF))
    maps = []
    for c in range(N_CORES):
        xT_own = np.ascontiguousarray(x_pad[c * cfg.NPC : (c + 1) * cfg.NPC].T)
        maps.append(dict(
            x_lo=x_lo, x_hi=x_hi, xT_own=xT_own,
            idxlo=idxlo[c], idxhi=idxhi[c], sall=sall[c],
            iden=iden,
            w1l=w1l_f, w1r=w1r_f, c1=c1, w2l=w2l,
        ))
    return maps


def make_inmaps2(cfg, idxlo, idxhi, sall, hw2l_full, hT_parts, W2_r, b2):
    iden = np.eye(P, dtype=np.float32).astype(BF)
    w2r = np.ascontiguousarray(np.asarray(W2_r, np.float32).astype(BF))
    b2c = np.asarray(b2, np.float32).reshape(cfg.O, 1)
    hw_pad = np.zeros((cfg.NP, P), BF)
    hw_pad[:, : cfg.O] = hw2l_full
    hw_lo = np.ascontiguousarray(hw_pad[: cfg.NLO])
    hw_hi = np.ascontiguousarray(hw_pad[SPLIT : SPLIT + cfg.NHI]) \
        if cfg.NP > SPLIT else np.zeros((cfg.NHI, P), BF)
    maps = []
    for c in range(N_CORES):
        maps.append(dict(
            hw_lo=hw_lo, hw_hi=hw_hi, hT_own=hT_parts[c],
            idxlo=idxlo[c], idxhi=idxhi[c], sall=sall[c],
            iden=iden, w2r=w2r, b2=b2c,
        ))
    return maps


_cache = {}


def _get_programs(T_lo, T_hi):
    key = (tuple(int(t) for t in T_lo), tuple(int(t) for t in T_hi))
    if key not in _cache:
        _cache[key] = (build_k1(CFG, T_lo, T_hi), build_k2(CFG, T_lo, T_hi))
    return _cache[key]


def kernel(x, edge_index, W1_l, W1_r, b1, bn_gamma, bn_beta, bn_mean, bn_var,
           W2_l, W2_r, b2, _results=None):
    cfg = CFG
    T_lo, T_hi, idxlo, idxhi, sall = _preprocess(edge_index, cfg)
    nc1, nc2 = _get_programs(T_lo, T_hi)
    maps1 = make_inmaps1(cfg, idxlo, idxhi, sall, x, W1_l, W1_r, b1,
                         bn_gamma, bn_beta, bn_mean, bn_var, W2_l)
    r1 = run_bass_kernel_spmd(nc1, maps1, list(range(N_CORES)))
    hw2l_full = np.concatenate(
        [r1.results[c]["hw2l"] for c in range(N_CORES)], axis=0)
    hT_parts = [r1.results[c]["hT"] for c in range(N_CORES)]
    maps2 = make_inmaps2(cfg, idxlo, idxhi, sall, hw2l_full, hT_parts,
                         W2_r, b2)
    r2 = run_bass_kernel_spmd(nc2, maps2, list(range(N_CORES)))
    out = np.concatenate(
        [r2.results[c]["out_own"] for c in range(N_CORES)], axis=0)
    if _results is not None:
        _results.extend([r1, r2])
    return out[: cfg.N]


# revision 9
# speedup vs baseline: 2.7114x; 1.1914x over previous
"""2-layer GraphSAGE (mean aggr + BN(eval) + ReLU) on Trainium2, 8-core SPMD.

Strategy (graph/data parallel, dst-node sharding), v3:
  - Host: sort edges by dst, partition dst nodes into 8 ranges (49 chunks of
    128 nodes per core). Within each chunk, split edges into lo (src < 32768)
    and hi (src >= 32768) streams (dma_gather indices are signed int16), pad
    each stream to 128-edge tiles with per-chunk tile counts shared across
    cores (SPMD: one program, 8 cores). Gather indices are pre-wrapped into
    the dma_gather layout (idx i at [16g + i%16, i//16], 8 group replicas);
    streams are padded to GK-tile multiples with -1 indices (the SWDGE ucode
    strips trailing negatives for free).
  - All gather tables and matmul operands are bf16 (PSUM accumulates fp32).
  - Gather calls round-robin over 4 SWDGE queues so descriptor generation
    runs on all four Q7 core pairs concurrently (4x Pool throughput); a
    single hoisted num_idxs register avoids per-call RegisterMove WAR
    serialization on the Pool sequencer.
  - Selection tiles S[e, d] = (dst_local[e]==d) * invdeg[dst[e]] are built on
    the HOST in a partition-major layout sall[e, t*128+d] and streamed by
    plain DMA (2KB/partition/call) — no DVE work in the main loop.
  - Device layer 1 (per core, per 128-node chunk):
      PSUM aggT[ch, node] += G^T @ S  (PE, bf16; G dma_gathered per tile)
      h^T = relu(W1l'^T @ aggT + W1r'^T @ xT + c1)   (BN folded into W/c)
      also emit hW2l = (h @ W2_l) row-major bf16 for layer-2 gathers
  - Host: allgather hW2l parts, zero-pad 64->128 cols, split at 32768.
  - Device layer 2: same aggregation over hW2l rows + W2_r self term + b2.
"""

import numpy as np
import ml_dtypes

import concourse.bacc as bacc
import concourse.bass as bass
import concourse.mybir as mybir
import concourse.tile as tile
from concourse.bass_utils import run_bass_kernel_spmd

F32 = mybir.dt.float32
BF16 = mybir.dt.bfloat16
I16 = mybir.dt.int16
AF = mybir.ActivationFunctionType
OP = mybir.AluOpType
BF = ml_dtypes.bfloat16

N_CORES = 8
P = 128
SPLIT = 32768                           # int16 signed index limit
NQ = 4                                  # SWDGE queues (4 Q7 core pairs)


class Cfg:
    def __init__(self, n_nodes, c_in, c_hid, c_out, cpc):
        self.N = n_nodes
        self.C, self.H, self.O = c_in, c_hid, c_out
        self.CPC = cpc                  # 128-node chunks per core
        self.NPC = cpc * P              # nodes per core
        self.NP = self.NPC * N_CORES    # padded node count
        assert self.NP >= n_nodes
        self.NLO = min(self.NP, SPLIT)          # rows in lo table
        self.NHI = max(self.NP - SPLIT, 1)      # rows in hi table


CFG = Cfg(50000, 128, 128, 64, 49)      # NP = 50176
GK = 8                                  # edge tiles per dma_gather call
                                        # (>1024 idxs per call crashes HW)


def _wrap_idx(a):
    """[NC, 128, NT] int32 -> dma_gather wrapped [NC, 128, NT*8] int16.
    Per tile t, edge e: wrapped[16g + e%16, t*8 + e//16] = a[e, t]."""
    ncr, _, nt = a.shape
    w16 = (a.reshape(ncr, 8, 16, nt).transpose(0, 2, 3, 1)
           .reshape(ncr, 16, nt * 8))
    return np.tile(w16, (1, 8, 1)).astype(np.uint16).view(np.int16)


def _pad8(n):
    return (n + GK - 1) // GK * GK


def _preprocess(edge_index, cfg):
    """Sort/partition edges; split per chunk into lo/hi gather streams.
    Returns per-chunk tile counts (shared across cores), wrapped int16 index
    arrays (padded to GK-tile multiples with -1), and the host-built
    partition-major selection table sall [NC, 128, NTT_pad*128] bf16."""
    src = np.asarray(edge_index[0]).astype(np.int64)
    dst = np.asarray(edge_index[1]).astype(np.int64)
    order = np.argsort(dst, kind="stable")
    s_src = src[order].astype(np.int32)
    s_dst = dst[order].astype(np.int32)
    deg = np.bincount(dst, minlength=cfg.NP).astype(np.float32)
    invdeg = (1.0 / np.maximum(deg, 1.0)).astype(np.float32)
    bounds = np.searchsorted(s_dst, np.arange(0, cfg.NP + 1, P)).astype(np.int64)
    islo = s_src < SPLIT

    nlo = np.zeros((N_CORES, cfg.CPC), np.int64)
    nhi = np.zeros((N_CORES, cfg.CPC), np.int64)
    for c in range(N_CORES):
        for ci in range(cfg.CPC):
            g = c * cfg.CPC + ci
            e0, e1 = int(bounds[g]), int(bounds[g + 1])
            nlo[c, ci] = int(islo[e0:e1].sum())
            nhi[c, ci] = (e1 - e0) - nlo[c, ci]
    T_lo = ((nlo.max(axis=0) + P - 1) // P).astype(np.int64)
    T_hi = ((nhi.max(axis=0) + P - 1) // P).astype(np.int64)
    T_lo[(T_lo == 0) & (T_hi == 0)] = 1     # every chunk needs >=1 matmul
    NT_lo, NT_hi = int(T_lo.sum()), int(T_hi.sum())
    NT_lo_p, NT_hi_p = _pad8(NT_lo), _pad8(NT_hi)
    lo_start = np.zeros(cfg.CPC, np.int64)
    lo_start[1:] = np.cumsum(T_lo)[:-1]
    hi_start = np.zeros(cfg.CPC, np.int64)
    hi_start[1:] = np.cumsum(T_hi)[:-1]
    NTT = NT_lo_p + NT_hi_p

    src_lo = np.zeros((N_CORES, P, NT_lo_p), np.int32)
    src_hi = np.zeros((N_CORES, P, NT_hi_p), np.int32)
    ids = np.full((N_CORES, P, NTT), -1, np.int32)
    ivd = np.zeros((N_CORES, P, NTT), np.float32)
    for c in range(N_CORES):
        for ci in range(cfg.CPC):
            g = c * cfg.CPC + ci
            e0, e1 = int(bounds[g]), int(bounds[g + 1])
            if e1 == e0:
                continue
            es = s_src[e0:e1]
            ed = s_dst[e0:e1]
            m = es < SPLIT
            for sel, st, s_off, arr, base in (
                (m, lo_start, 0, src_lo, 0),
                (~m, hi_start, NT_lo_p, src_hi, SPLIT),
            ):
                vs = es[sel] - base
                vd = ed[sel]
                n = len(vs)
                if n == 0:
                    continue
                j = np.arange(n)
                t = st[ci] + j // P
                pp = j % P
                arr[c, pp, t] = vs
                ids[c, pp, s_off + t] = vd - (c * cfg.NPC + ci * P)
                ivd[c, pp, s_off + t] = invdeg[vd]
    ids_bf = np.where(ids >= 0, ids, 300).astype(BF)
    ivd_bf = ivd.astype(BF)
    return T_lo, T_hi, _wrap_idx(src_lo), _wrap_idx(src_hi), ids_bf, ivd_bf


def _mk_nc():
    return bacc.Bacc(
        "TRN2",
        target_bir_lowering=False,
        debug=False,
        enable_asserts=False,
        num_devices=N_CORES,
        num_swdge_queues=NQ,
    )


def _agg_chunks(nc, cfg, T_lo, T_hi, d_lotab, d_hitab, t_idxlo, t_idxhi,
                t_ids, t_ivd, t_iota, reg_n, gplo, gphi, sp, pA, width, lw,
                tail, close_group=True):
    """Shared aggregation loop: per chunk accumulate G^T @ S into PSUM over
    the lo then hi tile streams, then call tail(ci, ps_agg). Gather calls
    round-robin over the SWDGE queues; S tiles are built GK at a time on the
    DVE via two wide tensor_tensor ops with stride-0 broadcasts.
    `width` is PSUM rows, `lw` is valid lhsT cols per 128-col tile."""
    NT_lo = int(T_lo.sum())
    NT_hi = int(T_hi.sum())
    NT_lo_p, NT_hi_p = _pad8(NT_lo), _pad8(NT_hi)
    # cursor, gather tile, base, sel tile
    state = {"lo": [0, None, 0, None], "hi": [0, None, 0, None]}
    qcnt = [0]
    lo_pos = hi_pos = 0
    for ci in range(cfg.CPC):
        Tl, Th = int(T_lo[ci]), int(T_hi[ci])
        ntile = Tl + Th
        ps_agg = pA.tile([width, P], F32)
        kk = 0
        for stream, pos, Tc, NTp, d_tab, t_idx, gp, col_off in (
            ("lo", lo_pos, Tl, NT_lo_p, d_lotab, t_idxlo, gplo, 0),
            ("hi", hi_pos, Th, NT_hi_p, d_hitab, t_idxhi, gphi, NT_lo_p),
        ):
            st = state[stream]
            for k in range(Tc):
                t = pos + k
                if t == st[0]:
                    g_tile = gp.tile([P, GK * P], BF16, tag="g" + stream)
                    nc.gpsimd.dma_gather(
                        out_ap=g_tile[:].rearrange("p (t c) -> p t c", c=P),
                        in_ap=d_tab.ap()[:, :],
                        idxs_ap=t_idx[:, t * 8 : (t + GK) * 8],
                        num_idxs=GK * P,
                        num_idxs_reg=GK * P,
                        elem_size=P,
                        queue_num=qcnt[0] % NQ,
                    )
                    qcnt[0] += 1
                    s_grp = sp.tile([P, GK * P], BF16, tag="s" + stream)
                    s3 = s_grp[:].rearrange("p (t c) -> p t c", c=P)
                    ids_b = (t_ids[:, col_off + t : col_off + t + GK]
                             .unsqueeze(2).broadcast_to([P, GK, P]))
                    ivd_b = (t_ivd[:, col_off + t : col_off + t + GK]
                             .unsqueeze(2).broadcast_to([P, GK, P]))
                    iota_b = t_iota[:].unsqueeze(1).broadcast_to([P, GK, P])
                    nc.vector.tensor_tensor(out=s3, in0=iota_b, in1=ids_b,
                                            op=OP.is_equal)
                    nc.vector.tensor_tensor(out=s3, in0=s3, in1=ivd_b,
                                            op=OP.mult)
                    st[0], st[1], st[2], st[3] = t + GK, g_tile, t, s_grp
                off = (t - st[2]) * P
                nc.tensor.matmul(
                    out=ps_agg[:],
                    lhsT=st[1][:, off : off + lw],
                    rhs=st[3][:, off : off + P],
                    start=(kk == 0),
                    stop=(close_group and kk == ntile - 1),
                )
                kk += 1
        lo_pos += Tl
        hi_pos += Th
        tail(ci, ps_agg)


def build_k1(cfg, T_lo, T_hi, compile=True):
    """Layer 1: x -> hT_own [H, NPC] bf16, hw2l_own [NPC, O] bf16 row-major."""
    NT_lo_p = _pad8(int(T_lo.sum()))
    NT_hi_p = _pad8(int(T_hi.sum()))
    NTT = NT_lo_p + NT_hi_p
    C, H, O = cfg.C, cfg.H, cfg.O
    nc = _mk_nc()
    d_xlo = nc.dram_tensor("x_lo", (cfg.NLO, C), BF16, kind="ExternalInput")
    d_xhi = nc.dram_tensor("x_hi", (cfg.NHI, C), BF16, kind="ExternalInput")
    d_xT = nc.dram_tensor("xT_own", (C, cfg.NPC), BF16, kind="ExternalInput")
    d_ilo = nc.dram_tensor("idxlo", (P, NT_lo_p * 8), I16, kind="ExternalInput")
    d_ihi = nc.dram_tensor("idxhi", (P, NT_hi_p * 8), I16, kind="ExternalInput")
    d_ids = nc.dram_tensor("ids", (P, NTT), BF16, kind="ExternalInput")
    d_ivd = nc.dram_tensor("ivd", (P, NTT), BF16, kind="ExternalInput")
    d_iota = nc.dram_tensor("iota", (P, P), BF16, kind="ExternalInput")
    d_iden = nc.dram_tensor("iden", (P, P), BF16, kind="ExternalInput")
    d_w1l = nc.dram_tensor("w1l", (C, H), BF16, kind="ExternalInput")
    d_w1r = nc.dram_tensor("w1r", (C, H), BF16, kind="ExternalInput")
    d_c1 = nc.dram_tensor("c1", (H, 1), F32, kind="ExternalInput")
    d_w2l = nc.dram_tensor("w2l", (H, O), BF16, kind="ExternalInput")
    d_hT = nc.dram_tensor("hT", (H, cfg.NPC), BF16, kind="ExternalOutput")
    d_hw2l = nc.dram_tensor("hw2l", (cfg.NPC, O), BF16, kind="ExternalOutput")

    with tile.TileContext(nc) as tc:
        with (
            tc.tile_pool(name="const", bufs=1) as cp,
            tc.tile_pool(name="glo", bufs=6) as gplo,
            tc.tile_pool(name="ghi", bufs=6) as gphi,
            tc.tile_pool(name="sel", bufs=6) as sp,
            tc.tile_pool(name="work", bufs=4) as wp,
            tc.tile_pool(name="psA", bufs=2, space="PSUM") as pA,
            tc.tile_pool(name="psB", bufs=2, space="PSUM") as pB,
            tc.tile_pool(name="psC", bufs=2, space="PSUM") as pC,
            tc.tile_pool(name="psD", bufs=2, space="PSUM") as pD,
        ):
            def cload(name, d, shape, dt=F32):
                t = cp.tile(shape, dt, tag=name)
                nc.sync.dma_start(t[:], d.ap()[:, :])
                return t

            t_iden = cload("iden", d_iden, [P, P], BF16)
            t_iota = cload("iota", d_iota, [P, P], BF16)
            t_ids = cload("ids", d_ids, [P, NTT], BF16)
            t_ivd = cload("ivd", d_ivd, [P, NTT], BF16)
            t_w1l = cload("w1l", d_w1l, [C, H], BF16)
            t_w1r = cload("w1r", d_w1r, [C, H], BF16)
            t_c1 = cload("c1", d_c1, [H, 1])
            t_w2l = cload("w2l", d_w2l, [H, O], BF16)
            t_ilo = cload("ilo", d_ilo, [P, NT_lo_p * 8], I16)
            t_ihi = cload("ihi", d_ihi, [P, NT_hi_p * 8], I16)
            reg_n = None

            def tail(ci, ps_agg):
                agg_sb = wp.tile([P, P], BF16, tag="agg")
                nc.scalar.copy(out=agg_sb[:], in_=ps_agg[:])
                xT_sb = wp.tile([P, P], BF16, tag="xT")
                nc.sync.dma_start(xT_sb[:], d_xT.ap()[:, ci * P : (ci + 1) * P])
                ps_h = pB.tile([P, P], F32)
                nc.tensor.matmul(out=ps_h[:], lhsT=t_w1l[:], rhs=agg_sb[:],
                                 start=True, stop=False)
                nc.tensor.matmul(out=ps_h[:], lhsT=t_w1r[:], rhs=xT_sb[:],
                                 start=False, stop=True)
                hT_sb = wp.tile([P, P], BF16, tag="hT")
                nc.scalar.activation(out=hT_sb[:], in_=ps_h[:], func=AF.Relu,
                                     bias=t_c1[:, :1], scale=1.0)
                nc.sync.dma_start(d_hT.ap()[:, ci * P : (ci + 1) * P], hT_sb[:])

                ps_w = pC.tile([O, P], F32)
                nc.tensor.matmul(out=ps_w[:], lhsT=t_w2l[:], rhs=hT_sb[:],
                                 start=True, stop=True)
                wT_sb = wp.tile([O, P], BF16, tag="wT")
                nc.scalar.copy(out=wT_sb[:], in_=ps_w[:])
                ps_r = pD.tile([P, O], BF16)
                nc.tensor.transpose(out=ps_r[:], in_=wT_sb[:],
                                    identity=t_iden[:O, :O])
                rm_sb = wp.tile([P, O], BF16, tag="rm")
                nc.scalar.copy(out=rm_sb[:], in_=ps_r[:])
                nc.sync.dma_start(d_hw2l.ap()[ci * P : (ci + 1) * P, :], rm_sb[:])

            _agg_chunks(nc, cfg, T_lo, T_hi, d_xlo, d_xhi, t_ilo, t_ihi,
                        t_ids, t_ivd, t_iota, reg_n, gplo, gphi, sp, pA, C, P,
                        tail)

    if compile:
        nc.compile()
    return nc


def build_k2(cfg, T_lo, T_hi, compile=True):
    """Layer 2: hw2l (bf16, padded to 128 cols, lo/hi split) + hT_own ->
    out_own [NPC, O] f32 row-major."""
    NT_lo_p = _pad8(int(T_lo.sum()))
    NT_hi_p = _pad8(int(T_hi.sum()))
    NTT = NT_lo_p + NT_hi_p
    H, O = cfg.H, cfg.O
    nc = _mk_nc()
    d_hwlo = nc.dram_tensor("hw_lo", (cfg.NLO, P), BF16, kind="ExternalInput")
    d_hwhi = nc.dram_tensor("hw_hi", (cfg.NHI, P), BF16, kind="ExternalInput")
    d_hT = nc.dram_tensor("hT_own", (H, cfg.NPC), BF16, kind="ExternalInput")
    d_ilo = nc.dram_tensor("idxlo", (P, NT_lo_p * 8), I16, kind="ExternalInput")
    d_ihi = nc.dram_tensor("idxhi", (P, NT_hi_p * 8), I16, kind="ExternalInput")
    d_ids = nc.dram_tensor("ids", (P, NTT), BF16, kind="ExternalInput")
    d_ivd = nc.dram_tensor("ivd", (P, NTT), BF16, kind="ExternalInput")
    d_iota = nc.dram_tensor("iota", (P, P), BF16, kind="ExternalInput")
    d_iden = nc.dram_tensor("iden", (P, P), BF16, kind="ExternalInput")
    d_w2r = nc.dram_tensor("w2r", (H, O), BF16, kind="ExternalInput")
    d_b2 = nc.dram_tensor("b2", (O, 1), F32, kind="ExternalInput")
    d_out = nc.dram_tensor("out_own", (cfg.NPC, O), F32, kind="ExternalOutput")

    with tile.TileContext(nc) as tc:
        with (
            tc.tile_pool(name="const", bufs=1) as cp,
            tc.tile_pool(name="glo", bufs=6) as gplo,
            tc.tile_pool(name="ghi", bufs=6) as gphi,
            tc.tile_pool(name="sel", bufs=6) as sp,
            tc.tile_pool(name="work", bufs=4) as wp,
            tc.tile_pool(name="psA", bufs=3, space="PSUM") as pA,
            tc.tile_pool(name="psD", bufs=2, space="PSUM") as pD,
        ):
            def cload(name, d, shape, dt=F32):
                t = cp.tile(shape, dt, tag=name)
                nc.sync.dma_start(t[:], d.ap()[:, :])
                return t

            t_iden = cload("iden", d_iden, [P, P], BF16)
            t_iota = cload("iota", d_iota, [P, P], BF16)
            t_ids = cload("ids", d_ids, [P, NTT], BF16)
            t_ivd = cload("ivd", d_ivd, [P, NTT], BF16)
            t_w2r = cload("w2r", d_w2r, [H, O], BF16)
            t_b2 = cload("b2", d_b2, [O, 1])
            t_ilo = cload("ilo", d_ilo, [P, NT_lo_p * 8], I16)
            t_ihi = cload("ihi", d_ihi, [P, NT_hi_p * 8], I16)
            t_hT = cp.tile([H, cfg.NPC], BF16, tag="hT_all")
            nc.sync.dma_start(t_hT[:], d_hT.ap()[:, :])
            reg_n = None

            def tail(ci, ps_agg):
                nc.tensor.matmul(
                    out=ps_agg[:],
                    lhsT=t_w2r[:],
                    rhs=t_hT[:, ci * P : (ci + 1) * P],
                    start=False,
                    stop=True,
                )
                oT_sb = wp.tile([O, P], BF16, tag="oT")
                nc.scalar.activation(out=oT_sb[:], in_=ps_agg[:],
                                     func=AF.Identity, bias=t_b2[:, :1], scale=1.0)
                ps_r = pD.tile([P, O], BF16)
                nc.tensor.transpose(out=ps_r[:], in_=oT_sb[:],
                                    identity=t_iden[:O, :O])
                rm_sb = wp.tile([P, O], F32, tag="rm")
                nc.scalar.copy(out=rm_sb[:], in_=ps_r[:])
                nc.sync.dma_start(d_out.ap()[ci * P : (ci + 1) * P, :], rm_sb[:])

            _agg_chunks(nc, cfg, T_lo, T_hi, d_hwlo, d_hwhi, t_ilo, t_ihi,
                        t_ids, t_ivd, t_iota, reg_n, gplo, gphi, sp, pA, O, O,
                        tail, close_group=False)

    if compile:
        nc.compile()
    return nc


def make_inmaps1(cfg, idxlo, idxhi, ids_bf, ivd_bf, x, W1_l, W1_r, b1,
                 bn_gamma, bn_beta, bn_mean, bn_var, W2_l):
    s = (np.asarray(bn_gamma, np.float64)
         / np.sqrt(np.asarray(bn_var, np.float64) + 1e-5))
    w1l_f = (np.asarray(W1_l, np.float64) * s[None, :]).astype(BF)
    w1r_f = (np.asarray(W1_r, np.float64) * s[None, :]).astype(BF)
    c1 = ((np.asarray(b1, np.float64) - np.asarray(bn_mean, np.float64)) * s
          + np.asarray(bn_beta, np.float64)).astype(np.float32).reshape(cfg.H, 1)
    x_pad = np.zeros((cfg.NP, cfg.C), BF)
    x_pad[: cfg.N] = np.asarray(x, np.float32).astype(BF)
    x_lo = np.ascontiguousarray(x_pad[: cfg.NLO])
    x_hi = np.ascontiguousarray(x_pad[SPLIT : SPLIT + cfg.NHI]) \
        if cfg.NP > SPLIT else np.zeros((cfg.NHI, cfg.C), BF)
    iden = np.eye(P, dtype=np.float32).astype(BF)
    iota = np.tile(np.arange(P, dtype=np.float32), (P, 1)).astype(BF)
    w2l = np.ascontiguousarray(np.asarray(W2_l, np.float32).astype(BF))
    maps = []
    for c in range(N_CORES):
        xT_own = np.ascontiguousarray(x_pad[c * cfg.NPC : (c + 1) * cfg.NPC].T)
        maps.append(dict(
            x_lo=x_lo, x_hi=x_hi, xT_own=xT_own,
            idxlo=idxlo[c], idxhi=idxhi[c], ids=ids_bf[c], ivd=ivd_bf[c],
            iota=iota, iden=iden,
            w1l=w1l_f, w1r=w1r_f, c1=c1, w2l=w2l,
        ))
    return maps


def make_inmaps2(cfg, idxlo, idxhi, ids_bf, ivd_bf, hw2l_full, hT_parts,
                 W2_r, b2):
    iden = np.eye(P, dtype=np.float32).astype(BF)
    iota = np.tile(np.arange(P, dtype=np.float32), (P, 1)).astype(BF)
    w2r = np.ascontiguousarray(np.asarray(W2_r, np.float32).astype(BF))
    b2c = np.asarray(b2, np.float32).reshape(cfg.O, 1)
    hw_pad = np.zeros((cfg.NP, P), BF)
    hw_pad[:, : cfg.O] = hw2l_full
    hw_lo = np.ascontiguousarray(hw_pad[: cfg.NLO])
    hw_hi = np.ascontiguousarray(hw_pad[SPLIT : SPLIT + cfg.NHI]) \
        if cfg.NP > SPLIT else np.zeros((cfg.NHI, P), BF)
    maps = []
    for c in range(N_CORES):
        maps.append(dict(
            hw_lo=hw_lo, hw_hi=hw_hi, hT_own=hT_parts[c],
            idxlo=idxlo[c], idxhi=idxhi[c], ids=ids_bf[c], ivd=ivd_bf[c],
            iota=iota, iden=iden, w2r=w2r, b2=b2c,
        ))
    return maps


_cache = {}


def _get_programs(T_lo, T_hi):
    key = (tuple(int(t) for t in T_lo), tuple(int(t) for t in T_hi))
    if key not in _cache:
        _cache[key] = (build_k1(CFG, T_lo, T_hi), build_k2(CFG, T_lo, T_hi))
    return _cache[key]


def kernel(x, edge_index, W1_l, W1_r, b1, bn_gamma, bn_beta, bn_mean, bn_var,
           W2_l, W2_r, b2, _results=None):
    cfg = CFG
    T_lo, T_hi, idxlo, idxhi, ids_bf, ivd_bf = _preprocess(edge_index, cfg)
    nc1, nc2 = _get_programs(T_lo, T_hi)
    maps1 = make_inmaps1(cfg, idxlo, idxhi, ids_bf, ivd_bf, x, W1_l, W1_r, b1,
                         bn_gamma, bn_beta, bn_mean, bn_var, W2_l)
    r1 = run_bass_kernel_spmd(nc1, maps1, list(range(N_CORES)))
    hw2l_full = np.concatenate(
        [r1.results[c]["hw2l"] for c in range(N_CORES)], axis=0)
    hT_parts = [r1.results[c]["hT"] for c in range(N_CORES)]
    maps2 = make_inmaps2(cfg, idxlo, idxhi, ids_bf, ivd_bf, hw2l_full,
                         hT_parts, W2_r, b2)
    r2 = run_bass_kernel_spmd(nc2, maps2, list(range(N_CORES)))
    out = np.concatenate(
        [r2.results[c]["out_own"] for c in range(N_CORES)], axis=0)
    if _results is not None:
        _results.extend([r1, r2])
    return out[: cfg.N]
